# revision 1
# baseline (speedup 1.0000x reference)
"""CMLANet Trainium2 kernel: 8-core SPMD, time-sharded with halos.

All GRU scans use scan-accelerated fixed-point iteration: gates are
computed from the previous iterate with dense matmuls, then the linear
recurrence h_t = z_t*h_{t-1} + (1-z_t)*n_t is applied exactly with the
hardware prefix-scan (tensor_tensor_scan).  The sequence dim is sharded
over 8 cores with a halo long enough that a zero initial state
converges.  Core 0's halo gates are hard-masked (r=z=0 via -1e9
pre-sigmoid, n=0 via bias cancellation) so the real sequence starts
from exactly h0=0.

The two 40-dim attention GRUs (a/o) are stacked on partitions 0:80 and
run as one block-diagonal GRU.  The attention softmax+context uses a
single merged AllReduce carrying unnormalized exp-score contexts plus
the local sums of exp.  The 80 (u,k) bilinear tensors for the second
memory pass are topic-sharded across cores (5 "a"-type + 5 "o"-type
pairs each, prefetched in bf16 during the main GRU) and the small
per-batch um1 result is AllGathered.
"""

import os
import sys
import numpy as np

sys.path.insert(0, "/opt/trn_rl_repo")

import concourse.bass as bass  # noqa: E402,F401
import concourse.bacc as bacc  # noqa: E402
import concourse.tile as tile  # noqa: E402
from concourse import mybir  # noqa: E402
from concourse.bass_utils import run_bass_kernel_spmd  # noqa: E402

import ml_dtypes  # noqa: E402

BF16NP = ml_dtypes.bfloat16

F32 = mybir.dt.float32
F32R = mybir.dt.float32r
BF16 = mybir.dt.bfloat16
ALU = mybir.AluOpType
AF = mybir.ActivationFunctionType
AX = mybir.AxisListType

BS, T, CS, DE, NH, NT, NC = 4, 2048, 5, 300, 512, 20, 5
NV = 2 * NT          # 40
P80 = 2 * NV         # 80 (stacked a/o)
NIN = DE * CS        # 1500
KIN = 12             # input feature chunks (1536 padded)
G = 3 * NH           # 1536
NGC = 12             # gate chunks of 128
NJC = 4              # hidden chunks of 128
NCORE = 8
CHUNK = T // NCORE   # 256
HALO_M = int(os.environ.get("CMLA_HALO_M", "64"))
TM = CHUNK + HALO_M
HALO_S = int(os.environ.get("CMLA_HALO_S", "64"))
TS = CHUNK + HALO_S
ITERS_MAIN = int(os.environ.get("CMLA_ITM", "4"))
FENCE = int(os.environ.get("CMLA_FENCE", "3"))
ITERS_SMALL = int(os.environ.get("CMLA_ITS", "3"))
TPOFF = HALO_M - HALO_S + 1   # H col of small-GRU local t=0
NUK = 10                      # (u,k) pairs per core

NEG = -1.0e9


def _tchunks(tm):
    cs, t0 = [], 0
    while t0 < tm:
        tw = min(128, tm - t0)
        cs.append((t0, tw))
        t0 += tw
    return cs


TCS = _tchunks(TM)
STCS = _tchunks(TS)

# (u,k) shard order: core c owns A-pool[5c:5c+5] + O-pool[5c:5c+5] where
# A-pool = stacked uk indices for (Ua, Vo) = [0:20)+[60:80) (use m_a) and
# O-pool = (Va, Uo) = [20:40)+[40:60) (use m_o).  Local uk 0:5 -> s=0,
# 5:10 -> s=1 uniformly on every core; the gather reassembles per-core
# halves back to stacked order.
A_POOL = list(range(0, 20)) + list(range(60, 80))
O_POOL = list(range(20, 40)) + list(range(40, 60))


def _dest_a(c):
    return 5 * c if c < 4 else 5 * c + 40


def _dest_o(c):
    return 5 * c + 20


_CACHE = {}


def _mm(nc, out, lhsT, rhs, start, stop):
    nc.tensor.matmul(out, lhsT, rhs, start=start, stop=stop)


def build_program(debug=False):
    nc = bacc.Bacc("TRN2", target_bir_lowering=False, debug=False,
                   num_devices=NCORE)

    def din(name, shape, dt=F32):
        return nc.dram_tensor(name, list(shape), dt,
                              kind="ExternalInput").ap()

    d = {}
    d["xT"] = din("xT", [KIN, 128, BS, TM], BF16)
    d["wihT"] = din("wihT", [KIN, 128, NGC, 128], BF16)
    d["whhT"] = din("whhT", [NJC, 128, NGC, 128])
    d["biasc"] = din("biasc", [128, NGC])
    d["bhhn"] = din("bhhn", [128, NJC])
    d["gneg"] = din("gneg", [128, 6, BS, HALO_M])
    d["sneg"] = din("sneg", [P80, 2, BS, HALO_S])
    d["um0s"] = din("um0s", [128, NJC, P80])
    d["gwihs"] = din("gwihs", [P80, 3 * P80])
    d["gwhhs"] = din("gwhhs", [P80, 3 * P80])
    d["sbias"] = din("sbias", [P80, 4])
    d["i80"] = din("i80", [P80, P80])
    d["vavos"] = din("vavos", [P80, 2])
    d["tml"] = din("tml", [8, CHUNK])
    d["sel"] = din("sel", [8, 32])
    d["tmv32"] = din("tmv32", [128, 32])
    d["ustat_sh"] = din("ustat_sh", [NUK, NJC, 128, NJC, 128], BF16)
    d["lalos"] = din("lalos", [P80, 10])
    d["lalobs"] = din("lalobs", [5, 2])
    d["ones5"] = din("ones5", [5, 1])

    d["y"] = nc.dram_tensor("y", [10, BS, CHUNK], F32,
                            kind="ExternalOutput").ap()
    if debug:
        d["hdbg"] = nc.dram_tensor("hdbg", [NJC, 128, BS, TM], F32,
                                   kind="ExternalOutput").ap()
        d["hadbg"] = nc.dram_tensor("hadbg", [P80, BS, TS], F32,
                                    kind="ExternalOutput").ap()

    d["fence1"] = nc.dram_tensor("fence1", [NJC, 128, BS, TM],
                                 F32).ap()
    d["fence2"] = nc.dram_tensor("fence2", [P80, BS, TS], F32).ap()
    d["cc_att_in"] = nc.dram_tensor("cc_att_in", [128, 33], F32)
    d["cc_att_out"] = nc.dram_tensor("cc_att_out", [NCORE, 128, 33], F32,
                                     addr_space="Shared")
    d["cc_um_in"] = nc.dram_tensor("cc_um_in", [128, NUK * 16], BF16)
    d["cc_um_out"] = nc.dram_tensor("cc_um_out", [NCORE, 128, NUK * 16],
                                    BF16, addr_space="Shared")

    with tile.TileContext(nc, num_cores=NCORE) as tc:
        _body(nc, tc, d, debug)
    nc.compile()
    return nc


def _body(nc, tc, d, debug):
    from contextlib import ExitStack
    es = ExitStack()
    ex = es.enter_context(tc.tile_pool(name="persist", bufs=1))

    H = [ex.tile([128, BS, TM + 1], F32R, name=f"H{j}") for j in range(NJC)]
    for j in range(NJC):
        nc.vector.memset(H[j][:, :, :].bitcast(F32), 0.0)

    # ---- persistent smalls + bf16 ustat shard (prefetch via Pool queue,
    # overlapping the xw phase + main DEER) ----
    ex2 = es.enter_context(tc.tile_pool(name="persist2", bufs=1))
    ust = ex2.tile([128, NUK, NJC, NJC, 128], BF16, name="ustt")
    for uk in range(NUK):
        nc.gpsimd.dma_start(
            ust[:, uk, :, :, :],
            d["ustat_sh"][uk].rearrange("j p h q -> p j h q"))
    um0s = ex2.tile([128, NJC, P80], F32R, name="um0st")
    gwihs = ex2.tile([P80, 3 * P80], F32R, name="gwihst")
    gwhhs = ex2.tile([P80, 3 * P80], F32R, name="gwhhst")
    sbias = ex2.tile([P80, 4], F32, name="sbiast")
    i80 = ex2.tile([P80, P80], F32R, name="i80t")
    vavos = ex2.tile([P80, 2], F32R, name="vavost")
    tml = ex2.tile([8, CHUNK], F32, name="tmlt")
    sel = ex2.tile([8, 32], F32R, name="selt")
    tmv32 = ex2.tile([128, 32], F32, name="tmv32t")
    sneg = ex2.tile([P80, 2, BS, HALO_S], F32, name="snegt")
    lalos = ex2.tile([P80, 10], F32R, name="lalost")
    lalobs = ex2.tile([5, 2], F32, name="lalobst")
    ones5 = ex2.tile([5, 1], F32R, name="ones5t")
    nc.gpsimd.dma_start(um0s[:, :, :], d["um0s"].bitcast(F32R))
    nc.gpsimd.dma_start(gwihs[:, :], d["gwihs"].bitcast(F32R))
    nc.gpsimd.dma_start(gwhhs[:, :], d["gwhhs"].bitcast(F32R))
    nc.gpsimd.dma_start(sbias[:, :], d["sbias"])
    nc.gpsimd.dma_start(i80[:, :], d["i80"].bitcast(F32R))
    nc.gpsimd.dma_start(vavos[:, :], d["vavos"].bitcast(F32R))
    nc.gpsimd.dma_start(tml[:, :], d["tml"])
    nc.gpsimd.dma_start(sel[:, :], d["sel"].bitcast(F32R))
    nc.gpsimd.dma_start(tmv32[:, :], d["tmv32"])
    nc.gpsimd.dma_start(sneg[:, :, :, :], d["sneg"])
    nc.gpsimd.dma_start(lalos[:, :], d["lalos"].bitcast(F32R))
    nc.gpsimd.dma_start(lalobs[:, :], d["lalobs"])
    nc.gpsimd.dma_start(ones5[:, :], d["ones5"].bitcast(F32R))

    # ======================= stages 1+2: main GRU =======================
    with tc.tile_pool(name="xwpool", bufs=1) as xp:
        xw_rz = [xp.tile([128, BS, TM], BF16, name=f"xwrz{g}")
                 for g in range(8)]
        xw_n = [xp.tile([128, BS, TM], BF16, name=f"xwn{j}")
                for j in range(NJC)]
        biasc = xp.tile([128, NGC], F32, name="biasct")
        bhhn = xp.tile([128, NJC], F32, name="bhhnt")
        nc.sync.dma_start(biasc[:, :], d["biasc"])
        nc.sync.dma_start(bhhn[:, :], d["bhhn"])

        # ---- xw = x @ Wih^T (+bih, +bhh folded for r,z) ----
        with tc.tile_pool(name="xstream", bufs=1) as stx:
            xs = stx.tile([128, KIN, BS, TM], BF16, name="xsfull")
            for (t0, tw) in TCS:
                for kc in range(KIN):
                    nc.sync.dma_start(xs[:, kc, :, t0:t0 + tw],
                                      d["xT"][kc, :, :, t0:t0 + tw])
            with tc.tile_pool(name="wstream", bufs=2) as st, \
                    tc.tile_pool(name="ps_xw", bufs=6, space="PSUM") as ps:
                for g in range(NGC):
                    ws = st.tile([128, KIN, 128], BF16, name="ws", tag="ws")
                    nc.scalar.dma_start(
                        ws[:, :, :],
                        d["wihT"][:, :, g, :].rearrange("k p q -> p k q"))
                    for (t0, tw) in TCS:
                        p = ps.tile([128, BS, tw], F32, name="xwp",
                                    tag="xwp")
                        for kc in range(KIN):
                            _mm(nc, p[:, :, :], ws[:, kc, :],
                                xs[:, kc, :, t0:t0 + tw],
                                start=(kc == 0), stop=(kc == KIN - 1))
                        o = xw_rz[g] if g < 8 else xw_n[g - 8]
                        nc.scalar.activation(o[:, :, t0:t0 + tw],
                                             p[:, :, :], AF.Identity,
                                             bias=biasc[:, g:g + 1])
        # core-0 halo gate hard-masking (other cores add zeros)
        with tc.tile_pool(name="gnegp", bufs=1) as gp:
            gneg = gp.tile([128, 6, BS, HALO_M], F32, name="gnegt")
            nc.sync.dma_start(gneg[:, :, :, :], d["gneg"])
            for j in range(NJC):
                nc.vector.tensor_add(
                    xw_rz[j][:, :, :HALO_M],
                    xw_rz[j][:, :, :HALO_M], gneg[:, 0])
                nc.vector.tensor_add(
                    xw_rz[4 + j][:, :, :HALO_M],
                    xw_rz[4 + j][:, :, :HALO_M], gneg[:, 1])
                nc.vector.tensor_add(
                    xw_n[j][:, :, :HALO_M], xw_n[j][:, :, :HALO_M],
                    gneg[:, 2 + j])

        # ---- scan-DEER iterations ----
        with tc.tile_pool(name="sc_deer", bufs=2) as sc, \
                tc.tile_pool(name="ps_deer", bufs=2, space="PSUM") as ps:
            whh = sc.tile([128, NJC, NGC, 128], F32R, name="whh")
            for j in range(NJC):
                nc.sync.dma_start(whh[:, j, :, :],
                                  d["whhT"][j].bitcast(F32R))
            for it in range(ITERS_MAIN):
                for (t0, tw) in TCS:
                    zc = [None] * NJC
                    bc = [None] * NJC
                    for j in range(NJC):
                        pr = ps.tile([128, BS, tw], F32, name="pr",
                                     tag="pr")
                        pz = ps.tile([128, BS, tw], F32, name="pz",
                                     tag="pz")
                        pn = ps.tile([128, BS, tw], F32, name="pn",
                                     tag="pn")
                        for jc in range(NJC):
                            hs = H[jc][:, :, t0:t0 + tw]
                            _mm(nc, pr[:, :, :], whh[:, jc, j, :], hs,
                                start=(jc == 0), stop=(jc == NJC - 1))
                            _mm(nc, pz[:, :, :], whh[:, jc, 4 + j, :], hs,
                                start=(jc == 0), stop=(jc == NJC - 1))
                            _mm(nc, pn[:, :, :], whh[:, jc, 8 + j, :], hs,
                                start=(jc == 0), stop=(jc == NJC - 1))
                        rs = sc.tile([128, BS, tw], F32, name="rs",
                                     tag="rs")
                        nc.vector.scalar_tensor_tensor(
                            rs[:, :, :], pr[:, :, :], 1.0,
                            xw_rz[j][:, :, t0:t0 + tw],
                            ALU.mult, ALU.add)
                        zs = sc.tile([128, BS, tw], F32, name="zs",
                                     tag="zs")
                        nc.vector.scalar_tensor_tensor(
                            zs[:, :, :], pz[:, :, :], 1.0,
                            xw_rz[4 + j][:, :, t0:t0 + tw],
                            ALU.mult, ALU.add)
                        r = sc.tile([128, BS, tw], F32, name="r", tag="r")
                        nc.scalar.activation(r[:, :, :], rs[:, :, :],
                                             AF.Sigmoid)
                        z = sc.tile([128, BS, tw], F32, name="z", tag="z")
                        nc.scalar.activation(z[:, :, :], zs[:, :, :],
                                             AF.Sigmoid)
                        rhn = sc.tile([128, BS, tw], F32, name="rhn",
                                      tag="rhn")
                        nc.vector.scalar_tensor_tensor(
                            rhn[:, :, :], pn[:, :, :], bhhn[:, j:j + 1],
                            r[:, :, :], ALU.add, ALU.mult)
                        npre = sc.tile([128, BS, tw], F32, name="npre",
                                       tag="npre")
                        nc.gpsimd.tensor_add(npre[:, :, :], rhn[:, :, :],
                                             xw_n[j][:, :, t0:t0 + tw])
                        nt_ = sc.tile([128, BS, tw], F32, name="nt",
                                      tag="nt")
                        nc.scalar.activation(nt_[:, :, :], npre[:, :, :],
                                             AF.Tanh)
                        iz = sc.tile([128, BS, tw], F32, name="iz",
                                     tag="iz")
                        nc.scalar.activation(iz[:, :, :], zs[:, :, :],
                                             AF.Sigmoid, scale=-1.0)
                        bm = sc.tile([128, BS, tw], F32, name="bmt",
                                     tag="bmt")
                        nc.gpsimd.tensor_mul(bm[:, :, :], iz[:, :, :],
                                             nt_[:, :, :])
                        zc[j], bc[j] = z, bm
                    for j in range(NJC):
                        for b in range(BS):
                            nc.vector.tensor_tensor_scan(
                                H[j][:, b, t0 + 1:t0 + tw + 1],
                                zc[j][:, b, :], bc[j][:, b, :],
                                H[j][:, b, t0:t0 + 1], ALU.mult,
                                ALU.add)

        if debug:
            for j in range(NJC):
                nc.sync.dma_start(d["hdbg"][j],
                                  H[j][:, :, 1:TM + 1].bitcast(F32))
        elif FENCE & 1:
            for j in range(NJC):
                nc.sync.dma_start(d["fence1"][j],
                                  H[j][:, :, 1:TM + 1].bitcast(F32))

    # ================== stages 3..7: attention + smalls ==================
    with tc.tile_pool(name="smallp", bufs=1) as sp:
        xwr = [sp.tile([P80, BS, TS], F32R, name=f"xwr{p}")
               for p in range(2)]
        xwz = [sp.tile([P80, BS, TS], F32R, name=f"xwz{p}")
               for p in range(2)]
        xwn = [sp.tile([P80, BS, TS], F32, name=f"xwn{p}")
               for p in range(2)]
        R = [sp.tile([P80, BS, TS + 1], F32R, name=f"R{p}")
             for p in range(2)]
        Hb = [sp.tile([128, BS, TS], BF16, name=f"Hb{hc}")
              for hc in range(NJC)]
        um1t = sp.tile([128, NJC, P80, BS], BF16, name="um1tt")
        m1b = sp.tile([128, 2, BS, NJC], BF16, name="m1bt")
        sco8 = sp.tile([8, CHUNK], F32, name="sco8")
        scc = [sp.tile([1, BS, 128], F32, name=f"scct{i}")
               for i in range(4)]
        esc = sp.tile([8, CHUNK], F32, name="esc")
        escm = sp.tile([8, CHUNK], F32, name="escm")
        rows = [sp.tile([1, CHUNK], F32, name=f"rowt{i}")
                for i in range(8)]
        sumloc = sp.tile([8, 1], F32, name="sumloc")
        cap = sp.tile([128, 33], F32, name="cap")
        car = sp.tile([128, 33], F32, name="car")
        m1 = sp.tile([128, 32], F32, name="m1t")
        um1loc = sp.tile([128, NJC, NUK, BS], BF16, name="um1loc")
        yt = [sp.tile([5, BS, CHUNK], F32, name=f"ytt{s}")
              for s in range(2)]
        # initialize every DMA-read staging tile in full (the race
        # checker flags DMA reads of bytes still attributed to tiles
        # from closed pools)
        nc.vector.memset(cap[:, :], 0.0)
        nc.vector.memset(escm[:, :], 0.0)
        nc.vector.memset(um1loc[:, :, :, :], 0.0)
        for i in range(4):
            nc.vector.memset(scc[i][:, :, :], 0.0)
        for s in range(2):
            nc.vector.memset(yt[s][:, :, :], 0.0)

        def small_deer(ph):
            with tc.tile_pool(name="sc_sd", bufs=2) as sc, \
                    tc.tile_pool(name="ps_sd", bufs=2, space="PSUM") as ps:
                for it in range(ITERS_SMALL):
                    for (t0, tw) in STCS:
                        p_r = ps.tile([P80, BS, tw], F32, name="p_r",
                                      tag="p_r")
                        p_z = ps.tile([P80, BS, tw], F32, name="p_z",
                                      tag="p_z")
                        p_n = ps.tile([P80, BS, tw], F32, name="p_n",
                                      tag="p_n")
                        _mm(nc, p_r[:, :, :], i80[:, :],
                            xwr[ph][:, :, t0:t0 + tw], start=True,
                            stop=False)
                        _mm(nc, p_r[:, :, :], gwhhs[:, 0:P80],
                            R[ph][:, :, t0:t0 + tw], start=False,
                            stop=True)
                        _mm(nc, p_z[:, :, :], i80[:, :],
                            xwz[ph][:, :, t0:t0 + tw], start=True,
                            stop=False)
                        _mm(nc, p_z[:, :, :], gwhhs[:, P80:2 * P80],
                            R[ph][:, :, t0:t0 + tw], start=False,
                            stop=True)
                        _mm(nc, p_n[:, :, :], gwhhs[:, 2 * P80:3 * P80],
                            R[ph][:, :, t0:t0 + tw], start=True,
                            stop=True)
                        r = sc.tile([P80, BS, tw], F32, name="sr",
                                    tag="sr")
                        nc.scalar.activation(r[:, :, :], p_r[:, :, :],
                                             AF.Sigmoid)
                        z = sc.tile([P80, BS, tw], F32, name="sz",
                                    tag="sz")
                        nc.scalar.activation(z[:, :, :], p_z[:, :, :],
                                             AF.Sigmoid)
                        rhn = sc.tile([P80, BS, tw], F32, name="srhn",
                                      tag="srhn")
                        nc.vector.scalar_tensor_tensor(
                            rhn[:, :, :], p_n[:, :, :], sbias[:, 3:4],
                            r[:, :, :], ALU.add, ALU.mult)
                        npre = sc.tile([P80, BS, tw], F32, name="snpre",
                                       tag="snpre")
                        nc.gpsimd.tensor_add(npre[:, :, :], rhn[:, :, :],
                                             xwn[ph][:, :, t0:t0 + tw])
                        nt_ = sc.tile([P80, BS, tw], F32, name="snt",
                                      tag="snt")
                        nc.scalar.activation(nt_[:, :, :], npre[:, :, :],
                                             AF.Tanh)
                        bm = sc.tile([P80, BS, tw], F32, name="sbm",
                                     tag="sbm")
                        nc.vector.scalar_tensor_tensor(
                            bm[:, :, :], z[:, :, :], 1.0, nt_[:, :, :],
                            ALU.subtract, ALU.mult)
                        for b in range(BS):
                            nc.vector.tensor_tensor_scan(
                                R[ph][:, b, t0 + 1:t0 + tw + 1],
                                z[:, b, :], bm[:, b, :],
                                R[ph][:, b, t0:t0 + 1], ALU.mult,
                                ALU.subtract)

        # ---- phase 1: a = tanh(h . Um0) stacked; gate inputs; DEER ----
        with tc.tile_pool(name="sc_tp1", bufs=2) as sc, \
                tc.tile_pool(name="ps_tp1", bufs=2, space="PSUM") as ps:
            for (t0, tw) in STCS:
                pa = ps.tile([P80, BS, tw], F32, name="pa", tag="pa")
                for hc in range(NJC):
                    hs = H[hc][:, :, TPOFF + t0:TPOFF + t0 + tw]
                    _mm(nc, pa[:, :, :], um0s[:, hc, :], hs,
                        start=(hc == 0), stop=(hc == NJC - 1))
                a = sc.tile([P80, BS, tw], F32R, name="a1", tag="a1")
                nc.scalar.activation(a[:, :, :], pa[:, :, :], AF.Tanh)
                for gi, xwg in enumerate((xwr[0], xwz[0], xwn[0])):
                    px = ps.tile([P80, BS, tw], F32, name="pxg",
                                 tag="pxg")
                    _mm(nc, px[:, :, :],
                        gwihs[:, gi * P80:(gi + 1) * P80], a[:, :, :],
                        start=True, stop=True)
                    nc.scalar.activation(
                        xwg[:, :, t0:t0 + tw], px[:, :, :],
                        AF.Identity, bias=sbias[:, gi:gi + 1])
            # h -> bf16 for phase-2 bilinear (overlaps small_deer 1)
            for hc in range(NJC):
                nc.scalar.copy(Hb[hc][:, :, :],
                               H[hc][:, :, TPOFF:TPOFF + TS])
        nc.vector.tensor_add(xwr[0][:, :, :HALO_S],
                             xwr[0][:, :, :HALO_S],
                             sneg[:, 0])
        nc.vector.tensor_add(xwz[0][:, :, :HALO_S],
                             xwz[0][:, :, :HALO_S],
                             sneg[:, 0])
        nc.vector.tensor_add(xwn[0][:, :, :HALO_S],
                             xwn[0][:, :, :HALO_S], sneg[:, 1])
        nc.vector.memset(R[0][:, :, :].bitcast(F32), 0.0)
        small_deer(0)

        if debug:
            nc.sync.dma_start(d["hadbg"], R[0][:, :, 1:TS + 1].bitcast(F32))
        elif FENCE & 2:
            nc.sync.dma_start(d["fence2"], R[0][:, :, 1:TS + 1].bitcast(F32))

        # ---- attention middle: one merged AllReduce ----
        with tc.tile_pool(name="sc_att", bufs=1) as sc, \
                tc.tile_pool(name="ps_att", bufs=2, space="PSUM") as ps:
            for ci, (t0, tw) in enumerate(((0, 128), (128, 128))):
                for s in range(2):
                    pv = ps.tile([1, BS, tw], F32, name="psc", tag="psc")
                    _mm(nc, pv[:, :, :], vavos[:, s:s + 1],
                        R[0][:, :, HALO_S + 1 + t0:HALO_S + 1 + t0 + tw],
                        start=True, stop=True)
                    nc.scalar.copy(scc[2 * ci + s][:, :, :], pv[:, :, :])
                    nc.sync.dma_start(sco8[4 * s:4 * s + 4, t0:t0 + tw],
                                      scc[2 * ci + s][0, :, :])
            nc.scalar.activation(esc[:, :], sco8[:, :], AF.Exp,
                                 accum_out=sumloc[:, 0:1])
            nc.vector.scalar_tensor_tensor(escm[:, :], esc[:, :], 1.0,
                                           tml[:, :], ALU.mult, ALU.mult)
            for s in range(2):
                for b in range(BS):
                    row = rows[4 * s + b]
                    nc.sync.dma_start(row[:, :],
                                      escm[4 * s + b:4 * s + b + 1, :])
                    ab = sc.tile([128, CHUNK], F32, name="ab", tag="ab")
                    nc.gpsimd.partition_broadcast(ab[:, :], row[0:1, :])
                    for hc in range(NJC):
                        prod = sc.tile([128, CHUNK], F32, name="prod",
                                       tag="prod")
                        nc.vector.scalar_tensor_tensor(
                            prod[:, :],
                            H[hc][:, b, HALO_M + 1:TM + 1].bitcast(F32),
                            1.0, ab[:, :], ALU.mult, ALU.mult,
                            accum_out=cap[:, 16 * s + 4 * b + hc:
                                          16 * s + 4 * b + hc + 1])
            nc.vector.tensor_copy(cap[0:8, 32:33], sumloc[:, 0:1])
            nc.sync.dma_start(d["cc_att_in"].ap(), cap[:, :])
            nc.gpsimd.collective_compute(
                "AllGather", ALU.bypass,
                replica_groups=[list(range(NCORE))],
                ins=[d["cc_att_in"].ap()], outs=[d["cc_att_out"].ap()])
            car8 = sc.tile([128, NCORE, 33], F32, name="car8")
            nc.sync.dma_start(car8[:, :, :], d["cc_att_out"].ap().rearrange(
                "c p f -> p c f"))
            nc.vector.tensor_add(car[:, :], car8[:, 0, :], car8[:, 1, :])
            for c in range(2, NCORE):
                nc.vector.tensor_add(car[:, :], car[:, :], car8[:, c, :])
            rcp = sc.tile([8, 1], F32, name="rcp")
            nc.vector.reciprocal(rcp[:, :], car[0:8, 32:33])
            rcpB = sc.tile([8, 128], F32R, name="rcpB")
            nc.vector.memset(rcpB[:, :].bitcast(F32), 1.0)
            nc.vector.tensor_scalar_mul(rcpB[:, :], rcpB[:, :],
                                        rcp[:, 0:1])
            pscl = ps.tile([128, 32], F32, name="pscl", tag="pscl")
            _mm(nc, pscl[:, :], rcpB[:, :], sel[:, :], start=True,
                stop=True)
            nc.vector.tensor_mul(m1[:, :], car[:, 0:32], pscl[:, :])
            nc.vector.tensor_add(m1[:, :], m1[:, :], tmv32[:, :])
            nc.vector.tensor_copy(
                m1b[:, :, :, :],
                m1[:, :].rearrange("p (s b h) -> p s b h", s=2, b=BS))

            # ---- local um1 for this core's 10 (u,k) pairs ----
            for uk in range(NUK):
                s_sel = 0 if uk < 5 else 1
                pu = ps.tile([128, NJC, BS], F32, name="pu", tag="pu")
                for hc in range(NJC):
                    for jc in range(NJC):
                        _mm(nc, pu[:, hc, :], ust[:, uk, jc, hc, :],
                            m1b[:, s_sel, :, jc],
                            start=(jc == 0), stop=(jc == NJC - 1))
                nc.scalar.copy(um1loc[:, :, uk, :], pu[:, :, :])
            nc.sync.dma_start(
                d["cc_um_in"].ap(),
                um1loc[:, :, :, :].rearrange("p h u b -> p (h u b)"))
            nc.gpsimd.collective_compute(
                "AllGather", ALU.bypass,
                replica_groups=[list(range(NCORE))],
                ins=[d["cc_um_in"].ap()], outs=[d["cc_um_out"].ap()])
            gv = d["cc_um_out"].ap().rearrange(
                "c p (h u b) -> c p h u b", h=NJC, u=NUK)
            qs = [nc.sync, nc.scalar, nc.gpsimd]
            for c in range(NCORE):
                da, do = _dest_a(c), _dest_o(c)
                qs[c % 3].dma_start(um1t[:, :, da:da + 5, :],
                                    gv[c, :, :, 0:5, :])
                qs[(c + 1) % 3].dma_start(um1t[:, :, do:do + 5, :],
                                          gv[c, :, :, 5:10, :])

        # ---- phase 2: per-b bilinear + gate inputs; DEER ----
        with tc.tile_pool(name="sc_tp2", bufs=2) as sc, \
                tc.tile_pool(name="ps_tp2", bufs=2, space="PSUM") as ps:
            for b in range(BS):
                pa = ps.tile([P80, TS], F32, name="pa2", tag="pa2")
                for hc in range(NJC):
                    _mm(nc, pa[:, :], um1t[:, hc, :, b], Hb[hc][:, b, :],
                        start=(hc == 0), stop=(hc == NJC - 1))
                a = sc.tile([P80, TS], F32R, name="a2", tag="a2")
                nc.scalar.activation(a[:, :], pa[:, :], AF.Tanh)
                for gi, xwg in enumerate((xwr[1], xwz[1], xwn[1])):
                    px = ps.tile([P80, TS], F32, name="pxg2", tag="pxg2")
                    _mm(nc, px[:, :],
                        gwihs[:, gi * P80:(gi + 1) * P80], a[:, :],
                        start=True, stop=True)
                    nc.scalar.activation(
                        xwg[:, b, :], px[:, :],
                        AF.Identity, bias=sbias[:, gi:gi + 1])
        nc.vector.tensor_add(xwr[1][:, :, :HALO_S],
                             xwr[1][:, :, :HALO_S],
                             sneg[:, 0])
        nc.vector.tensor_add(xwz[1][:, :, :HALO_S],
                             xwz[1][:, :, :HALO_S],
                             sneg[:, 0])
        nc.vector.tensor_add(xwn[1][:, :, :HALO_S],
                             xwn[1][:, :, :HALO_S], sneg[:, 1])
        nc.vector.memset(R[1][:, :, :].bitcast(F32), 0.0)
        small_deer(1)

        # ---- final: ha = R1+R2; logits; per-class softmax ----
        with tc.tile_pool(name="sc_fin", bufs=4) as sc, \
                tc.tile_pool(name="ps_fin", bufs=3, space="PSUM") as ps:
            ha = sc.tile([P80, BS, CHUNK], F32R, name="hat")
            nc.vector.tensor_add(
                ha[:, :, :],
                R[0][:, :, HALO_S + 1:TS + 1],
                R[1][:, :, HALO_S + 1:TS + 1])
            for ci, (t0, tw) in enumerate(((0, 128), (128, 128))):
                for s in range(2):
                    pl = ps.tile([5, BS, tw], F32, name="pl", tag="pl")
                    _mm(nc, pl[:, :, :], lalos[:, 5 * s:5 * s + 5],
                        ha[:, :, t0:t0 + tw], start=True, stop=True)
                    el = sc.tile([5, BS, tw], F32R, name="el", tag="el")
                    nc.scalar.activation(el[:, :, :],
                                         pl[:, :, :], AF.Exp,
                                         bias=lalobs[:, s:s + 1])
                    pss = ps.tile([1, BS, tw], F32, name="pss", tag="pss")
                    _mm(nc, pss[:, :, :], ones5[:, :],
                        el[:, :, :], start=True, stop=True)
                    rs = sc.tile([1, BS, tw], F32, name="rst", tag="rst")
                    nc.vector.reciprocal(rs[:, :, :], pss[:, :, :])
                    rb = sc.tile([5, BS, tw], F32, name="rb", tag="rb")
                    nc.gpsimd.partition_broadcast(rb[:, :, :],
                                                  rs[0:1, :, :],
                                                  channels=5)
                    nc.vector.tensor_mul(yt[s][:, :, t0:t0 + tw],
                                         el[:, :, :],
                                         rb[:, :, :])
            for s in range(2):
                nc.sync.dma_start(d["y"][5 * s:5 * s + 5], yt[s][:, :, :])
    es.close()


# ----------------------------------------------------------------------------
# host side
# ----------------------------------------------------------------------------

def _prep_inputs(inputs):
    inp = {k: (np.asarray(v) if not np.isscalar(v) else v)
           for k, v in inputs.items()}
    emb = np.asarray(inp["emb"], np.float32)
    idx = np.asarray(inp["index_embed"])
    cw = np.asarray(inp["context_words"])
    seq = int(np.asarray(inp["seq_size"]))

    tok = emb.T[idx]
    pad = np.broadcast_to(np.asarray(inp["padding"], np.float32),
                          (BS, 1, DE))
    pkt = np.broadcast_to(np.asarray(inp["punkt"], np.float32), (BS, 1, DE))
    nodes = np.concatenate([tok, pad, pkt], axis=1).astype(np.float32)
    x = np.stack([nodes[b][cw[b]] for b in range(BS)]).reshape(BS, T, NIN)
    xp = np.zeros((BS, T, KIN * 128), np.float32)
    xp[:, :, :NIN] = x

    Wih = np.asarray(inp["gru_Wih"], np.float32)
    Whh = np.asarray(inp["gru_Whh"], np.float32)
    bih = np.asarray(inp["gru_bih"], np.float32)
    bhh = np.asarray(inp["gru_bhh"], np.float32)

    wp = np.zeros((G, KIN * 128), np.float32)
    wp[:, :NIN] = Wih
    wihT = np.ascontiguousarray(
        wp.reshape(NGC, 128, KIN, 128).transpose(2, 3, 0, 1)).astype(BF16NP)
    whhT = np.ascontiguousarray(
        Whh.reshape(NGC, 128, NJC, 128).transpose(2, 3, 0, 1))
    biasc = np.zeros((128, NGC), np.float32)
    for g in range(NGC):
        biasc[:, g] = bih[g * 128:(g + 1) * 128]
        if g < 8:
            biasc[:, g] += bhh[g * 128:(g + 1) * 128]
    bhhn = np.ascontiguousarray(bhh[2 * NH:].reshape(NJC, 128).T)

    i80 = np.eye(P80, dtype=np.float32)

    Ua = np.asarray(inp["Ua"], np.float32)
    Va = np.asarray(inp["Va"], np.float32)
    Uo = np.asarray(inp["Uo"], np.float32)
    Vo = np.asarray(inp["Vo"], np.float32)
    m0a = np.asarray(inp["m0_a"], np.float32)
    m0o = np.asarray(inp["m0_o"], np.float32)
    ums = [np.einsum("khj,j->kh", U, m)
           for U, m in ((Ua, m0a), (Va, m0o), (Uo, m0o), (Vo, m0a))]
    um0s = np.zeros((128, NJC, P80), np.float32)
    for hc in range(NJC):
        for u in range(4):
            um0s[:, hc, u * NT:(u + 1) * NT] = \
                ums[u][:, hc * 128:(hc + 1) * 128].T

    gwihs = np.zeros((P80, 3 * P80), np.float32)
    gwhhs = np.zeros((P80, 3 * P80), np.float32)
    aWih = np.asarray(inp["ga_Wih"], np.float32)
    oWih = np.asarray(inp["go_Wih"], np.float32)
    aWhh = np.asarray(inp["ga_Whh"], np.float32)
    oWhh = np.asarray(inp["go_Whh"], np.float32)
    for gi in range(3):
        gwihs[0:NV, gi * P80:gi * P80 + NV] = \
            aWih[gi * NV:(gi + 1) * NV, :].T
        gwihs[NV:P80, gi * P80 + NV:(gi + 1) * P80] = \
            oWih[gi * NV:(gi + 1) * NV, :].T
        gwhhs[0:NV, gi * P80:gi * P80 + NV] = \
            aWhh[gi * NV:(gi + 1) * NV, :].T
        gwhhs[NV:P80, gi * P80 + NV:(gi + 1) * P80] = \
            oWhh[gi * NV:(gi + 1) * NV, :].T

    sbias = np.zeros((P80, 4), np.float32)
    for s, (bi_n, bh_n) in enumerate(
            (("ga_bih", "ga_bhh"), ("go_bih", "go_bhh"))):
        bi = np.asarray(inp[bi_n], np.float32)
        bh = np.asarray(inp[bh_n], np.float32)
        sl = slice(s * NV, (s + 1) * NV)
        sbias[sl, 0] = bi[:NV] + bh[:NV]
        sbias[sl, 1] = bi[NV:2 * NV] + bh[NV:2 * NV]
        sbias[sl, 2] = bi[2 * NV:]
        sbias[sl, 3] = bh[2 * NV:]

    vavos = np.zeros((P80, 2), np.float32)
    vavos[0:NV, 0] = np.asarray(inp["va"], np.float32)
    vavos[NV:P80, 1] = np.asarray(inp["vo"], np.float32)

    Ma = np.asarray(inp["Ma"], np.float32)
    Mo = np.asarray(inp["Mo"], np.float32)
    tma = np.tanh(m0a @ Ma).astype(np.float32)
    tmo = np.tanh(m0o @ Mo).astype(np.float32)
    tmv32 = np.zeros((128, 2, BS, NJC), np.float32)
    for hc in range(NJC):
        tmv32[:, 0, :, hc] = tma[hc * 128:(hc + 1) * 128][:, None]
        tmv32[:, 1, :, hc] = tmo[hc * 128:(hc + 1) * 128][:, None]
    tmv32 = np.ascontiguousarray(tmv32.reshape(128, 32))

    sel = np.zeros((8, 32), np.float32)
    for p in range(8):
        s, b = p // 4, p % 4
        for hc in range(NJC):
            sel[p, 16 * s + 4 * b + hc] = 1.0

    ustat = np.zeros((4 * NT, NJC, 128, NJC, 128), np.float32)
    for u, U in enumerate((Ua, Va, Uo, Vo)):
        ustat[u * NT:(u + 1) * NT] = U.reshape(
            NT, NJC, 128, NJC, 128).transpose(0, 3, 4, 1, 2)
    ustat = ustat.astype(BF16NP)

    lalos = np.zeros((P80, 10), np.float32)
    lalos[0:NV, 0:5] = np.asarray(inp["la_W"], np.float32).T
    lalos[NV:P80, 5:10] = np.asarray(inp["lo_W"], np.float32).T
    lalobs = np.ascontiguousarray(
        np.stack([np.asarray(inp["la_b"], np.float32),
                  np.asarray(inp["lo_b"], np.float32)], axis=1))
    ones5 = np.ones((5, 1), np.float32)

    shared = dict(wihT=wihT, whhT=whhT, biasc=biasc, bhhn=bhhn,
                  um0s=um0s, gwihs=gwihs, gwhhs=gwhhs, sbias=sbias,
                  i80=i80, vavos=vavos, sel=sel, tmv32=tmv32,
                  lalos=lalos, lalobs=lalobs, ones5=ones5)

    in_maps = []
    for c in range(NCORE):
        t0g = c * CHUNK - HALO_M
        xcm = np.zeros((BS, TM, KIN * 128), np.float32)
        lo = max(0, -t0g)
        xcm[:, lo:, :] = xp[:, t0g + lo:t0g + TM, :]
        xT = np.ascontiguousarray(
            xcm.transpose(2, 0, 1).reshape(KIN, 128, BS, TM)).astype(BF16NP)
        gneg = np.zeros((128, 6, BS, HALO_M), np.float32)
        sneg = np.zeros((P80, 2, BS, HALO_S), np.float32)
        if c == 0:
            gneg[:, 0] = NEG
            gneg[:, 1] = NEG
            bn = bih[2 * NH:].astype(BF16NP).astype(np.float32)
            for j in range(NJC):
                gneg[:, 2 + j] = -bn[j * 128:(j + 1) * 128][:, None, None]
            sneg[:, 0] = NEG
            sneg[:, 1] = -sbias[:, 2][:, None, None]
        tml_ = np.zeros((8, CHUNK), np.float32)
        w0, w1 = c * CHUNK, (c + 1) * CHUNK
        n_valid = max(0, min(seq, w1) - w0)
        tml_[:, :n_valid] = 1.0
        uks = A_POOL[5 * c:5 * c + 5] + O_POOL[5 * c:5 * c + 5]
        m = dict(shared)
        m.update(xT=xT, gneg=gneg, sneg=sneg, tml=tml_,
                 ustat_sh=np.ascontiguousarray(ustat[uks]))
        in_maps.append(m)
    return in_maps


def kernel(**inputs):
    debug = bool(int(os.environ.get("CMLA_DEBUG", "0")))
    key = ("prog", debug, FENCE)
    if key not in _CACHE:
        _CACHE[key] = build_program(debug=debug)
    nc = _CACHE[key]
    in_maps = _prep_inputs(inputs)
    res = run_bass_kernel_spmd(
        nc, in_maps, list(range(NCORE)),
        trace=bool(int(os.environ.get("CMLA_TRACE", "0"))))
    _CACHE["last_results"] = res
    ya = np.zeros((BS, T, NC), np.float32)
    yo = np.zeros((BS, T, NC), np.float32)
    for c in range(NCORE):
        y = res.results[c]["y"]
        ya[:, c * CHUNK:(c + 1) * CHUNK, :] = y[:5].transpose(1, 2, 0)
        yo[:, c * CHUNK:(c + 1) * CHUNK, :] = y[5:].transpose(1, 2, 0)
    return ya, yo

